# revision 34
# baseline (speedup 1.0000x reference)
"""Trainium2 Bass kernel for nn_AttnConv2d (attention-conv + dynamic conv + BN).

Math (per sample b):
  a1 = conv3x3(x, w1); a2 = conv3x3(x, w2); a3 = conv3x3(x, w3)     (SAME pad)
  attn[h,w,i,o] = sum_{p,q} a1[i,3p+h,3q+w] * a2[o,3p+h,3q+w]
  kern[o,:,:,:] = softmax(attn[.,.,.,o] / sqrt(Ci*9))
  av = conv3x3(a3, kern[b])                                         (per-sample kernel)
  y  = feature_map_stack(av)   (pure spatial/channel permutation)
  out = cm * x + NORM_SCALE * (y - mean_y) * rsqrt(var_y + eps)     (batch stats)

Sharding: data-parallel over batch, 1 sample per core, 8 cores.  The only
cross-core exchange is an AllReduce of the per-channel BN partial sums; a
warm-up collective fires early so the real one pays no setup cost.

Implementation notes:
  - a1/a2 convs run bf16 on the PE (f32 PSUM accumulate); the a3 conv and the
    dynamic per-sample conv run fp8e4 with perf_mode=DoubleRow: kernel-offset
    pairs are fed as the DR contraction pair (rhs pair stride = the spatial
    delta between the two offsets), so 9 offset matmuls become 4 DR + 1
    normal.  w3 is host-scaled x16 (undone in the PSUM->a3p copy), softmax
    output is scaled x128 into fp8 (undone in the PSUM->av copy); a3 is
    stored fp8 so the dynamic conv rhs reads it directly.
  - the DoubleRow rhs must be [K, 2, N] (one flat N dim), so conv rows are
    streamed as one flat 2*WP window: out row 0 lands in PSUM cols [0,W),
    row 1 at [WP,WP+W), 4 junk cols are skipped by the PSUM copy (xs8/a3p
    carry a zeroed junk-guard row for the last tile's overrun).
  - attention needs positions on the partition axis: a1g/a2g strips are
    transposed by the DMA XBAR (dma_start_transpose) instead of the PE; attn
    matmuls for strip s issue after the convs of strip s+1.  The LAST two
    strips' a3 convs are deferred past the final a1/a2 strip so their PE time
    covers the last strip's XBAR transposes.
  - av goes to DRAM through a channel-planar scratch avq[co, s, c2, t', q]:
    the partition-crossing half of feature_map_stack happens in the scatter
    (32 contiguous fp8 PQ-runs per DMA -- NOT the output layout, whose 192B
    runs cost ~50k DMA descriptors and starve the AllReduce's fabric packets
    of DMA-engine slots); the spatial (Y,X) interleave is undone for free by
    the pass-C engine APs.  The scratch is fp8 (quantization is inside the
    BN, so it costs ~1e-3 rel err and halves scatter/readback bytes).
  - x strips loaded for pass A are RETAINED in SBUF (pa_x bufs=NS) and reused
    as the pass-C residual, so pass C issues no x reloads at all.
  - a warm-up AllReduce fires at kernel START (input: a const tile): the CC
    engine's ~11us mesh cold-start and any cross-core launch skew get paid
    under pass A.  A warm-up right before the real AR is poison: collectives
    serialize on the CC core, so a late-arriving warm-up delays the real one.
  - the AllReduce payload is split: strips 0..NS-2 partial sums are reduced
    and DMA'd to the collective input under the last strip's compute; the
    trigger then only waits for strip NS-1's raw stats columns (no reduce),
    and the two halves are summed post-AR.
  - after pass B, a3p/kern are freed and the avq readback pool is allocated;
    readback DMAs issue under the AllReduce window.  Pass C applies
    t = sc*av + bb per half strip (Act Identity for 11 of 16 halves, DVE
    tensor_scalar for 5), then DVE adds the residual.  GpSimd must stay idle
    here: gpsimd SBUF ops lock DVE out of 2x-port mode.
"""

import os
import sys

for _p in ("/opt/trn_rl_repo", "/root/.axon_site/_ro/trn_rl_repo"):
    if os.path.isdir(_p) and _p not in sys.path:
        sys.path.insert(0, _p)
        break

import numpy as np

import concourse.bass as bass
import concourse.bacc as bacc
import concourse.tile as tile
from concourse import mybir

F32 = mybir.dt.float32
BF16 = mybir.dt.bfloat16
F8 = mybir.dt.float8e4
DR = mybir.MatmulPerfMode.DoubleRow

ATTN_K = 3
NH = 2
EPS = 1e-5
NORM_SCALE = 0.1816
CI = 128
CO = 128
W3S = 16.0     # host scale on w3 (fp8), undone in the PSUM->a3p copy
KSCL = 128.0   # scale on softmax output (fp8), undone in the PSUM->av copy

# DoubleRow offset pairing: (dy, dx, rhs pair stride as fn of WP)
# pairs are (k0,k1) (k2,k3) (k4,k5) (k6,k7) of the row-major 3x3 offsets,
# matching weight layout [128, 9, 128] sliced [:, 2p:2p+2, :]; k8 is single.
def _dr_pairs(WP):
    return [(0, 0, 1), (0, 2, WP - 2), (1, 1, 1), (2, 0, 1)]


def _rap(base, dims, off=0):
    """Raw AP on the same tensor as `base` (keeps base's partition dim)."""
    return bass.AP(tensor=base.tensor, offset=base.offset + off,
                   ap=[base.ap[0]] + [list(d) for d in dims])


def build_nc(H, W, R, n_cores, cm, level=5, pb_parity=True):
    """Build the per-core Bass kernel. R = strip rows (div by 6, even)."""
    assert H % R == 0 and R % 6 == 0 and W % 6 == 0
    NS = H // R                      # strips
    Wq = W // 3                      # attn subgrid cols
    P = (R // 3) * Wq                # attn positions per offset per strip
    S = H // 2                       # quadrant size of feature_map_stack
    NT = R // 2                      # psum tiles (2 rows) per strip
    Wh = W // 2
    PQ = NT * Wh                     # parity-split positions per strip
    N_TOT = float(n_cores * H * W)   # BN count per channel
    SCL = 1.0 / float(np.sqrt(CI * 9))
    WP = W + 2                       # padded row length
    assert P % 128 == 0
    NE = P // 128                    # 128-position chunks per offset per strip

    nc = bacc.Bacc("TRN2", target_bir_lowering=False, debug=False,
                   num_devices=n_cores)

    x_in = nc.dram_tensor("x", [128, H + 2, WP], BF16,
                          kind="ExternalInput").ap()   # host-padded (+1 ring)
    x8_in = nc.dram_tensor("x8", [128, H + 2, WP], F8,
                           kind="ExternalInput").ap()  # fp8 copy of x
    w1_in = nc.dram_tensor("w1t", [128, 9, 128], BF16, kind="ExternalInput").ap()
    w2_in = nc.dram_tensor("w2t", [128, 9, 128], BF16, kind="ExternalInput").ap()
    w3_in = nc.dram_tensor("w3t8", [128, 9, 128], F8, kind="ExternalInput").ap()
    id_in = nc.dram_tensor("ident", [128, 128], BF16, kind="ExternalInput").ap()
    gp_in = nc.dram_tensor("gsum", [128, 128], F32, kind="ExternalInput").ap()
    mk_in = nc.dram_tensor("mask4", [128, 4], F32, kind="ExternalInput").ap()
    out_d = nc.dram_tensor("out", [128, H, W], BF16, kind="ExternalOutput").ap()
    # channel-planar scratch: avq[co, s, c2, t', q] -- the partition-crossing
    # half of feature_map_stack happens in the scatter (32 contiguous PQ-runs
    # per DMA), the spatial (Y,X) reshuffle happens in pass-C engine APs.
    avq_d = nc.dram_tensor("avq", [128, H * W], F8).ap()

    with tile.TileContext(nc) as tc:
        consts = tc.alloc_tile_pool(name="consts", bufs=1)
        w1t = consts.tile([128, 9, 128], BF16, tag="w1t")
        w2t = consts.tile([128, 9, 128], BF16, tag="w2t")
        w3t8 = consts.tile([128, 9, 128], F8, tag="w3t8")
        ident = consts.tile([128, 128], BF16, tag="ident")
        gsum = consts.tile([128, 128], F32, tag="gsum")
        mask4 = consts.tile([128, 4], F32, tag="mask4")
        nc.sync.dma_start(out=w1t[:], in_=w1_in[:])

        small = tc.alloc_tile_pool(name="small", bufs=1)
        stats_cols = small.tile([128, NS, 4, 2], F32, tag="stats_cols")
        sglob = small.tile([128, 8], F32, tag="sglob")
        sglob_g = small.tile([128, 8], F32, tag="sglob_g")
        sglob2 = small.tile([128, 16], F32, tag="sglob2")
        scalars = small.tile([128, 16], F32, tag="scalars")
        msb = small.tile([128, 8], F32, tag="msb")
        sel = small.tile([128, 4], F32, tag="sel")

        # pa_x sits low in the pool stack: its strips are retained through
        # pass C (the residual), while kern/a3p above it release after pass B.
        pa_x = tc.alloc_tile_pool(name="pa_x", bufs=NS)

        kern_pool = tc.alloc_tile_pool(name="kern", bufs=1)
        kernT = kern_pool.tile([128, 9, 128], F8, tag="kT", name="kT")

        # +1 junk-guard row: the flat 2*WP-column DoubleRow rhs of the last
        # tile reads 2 elements past row H+1 (results land in skipped cols).
        a3_pool = tc.alloc_tile_pool(name="a3p", bufs=1)
        a3p = a3_pool.tile([128, H + 3, WP], F8, tag="a3p")
        # zero the pad border of a3p once
        nc.vector.memset(_rap(a3p[:], [[1, WP]]), 0.0)                      # row 0
        nc.vector.memset(_rap(a3p[:], [[1, 2 * WP]], (H + 1) * WP), 0.0)    # rows H+1,H+2
        nc.vector.memset(_rap(a3p[:], [[WP, H + 3]]), 0.0)                  # col 0
        nc.vector.memset(_rap(a3p[:], [[WP, H + 3]], W + 1), 0.0)           # col W+1

        attn_psp = tc.alloc_tile_pool(name="attn_ps", bufs=1, space="PSUM")
        attn_ps = attn_psp.tile([128, 9 * 128], F32, tag="attn")

        # ---------------- pass A: static convs + attention accumulation ------
        pa_x8 = tc.alloc_tile_pool(name="pa_x8", bufs=2)
        pa_g = tc.alloc_tile_pool(name="pa_g", bufs=2)
        # 2 tags x 18 bufs: tile (s,hw) recycles (s-2,hw)'s buffer, whose attn
        # MMs were emitted before strip s-1's transposes -- no stall.
        pa_t = tc.alloc_tile_pool(name="pa_t", bufs=18)
        pa_cps = tc.alloc_tile_pool(name="pa_cps", bufs=3, space="PSUM")
        pa_cps8 = tc.alloc_tile_pool(name="pa_cps8", bufs=2, space="PSUM")

        aT_tiles = {}

        def emit_attn_mms(s):
            a1T, a2T = aT_tiles.pop(s)
            for hw in range(9):
                for e in range(NE):
                    nc.tensor.matmul(
                        attn_ps[:, hw * 128:(hw + 1) * 128],
                        a2T[hw][:, e, :], a1T[hw][:, e, :],
                        start=(s == 0 and e == 0 and hw in (0, 4, 8)),
                        stop=(s == NS - 1 and e == NE - 1 and hw in (3, 7, 8)),
                        skip_group_check=True)

        xs_tiles = {}
        xs8_tiles = {}

        def load_xs(s):
            t = pa_x.tile([128, R + 2, WP], BF16, tag="xs")
            if s == 0:
                # split the first strip so tile-0 matmuls start ~3us earlier
                nc.sync.dma_start(out=t[:, 0:4, :], in_=x_in[:, 0:4, :])
                nc.sync.dma_start(out=t[:, 4:14, :], in_=x_in[:, 4:14, :])
                nc.sync.dma_start(out=t[:, 14:R + 2, :],
                                  in_=x_in[:, 14:R + 2, :])
            else:
                nc.sync.dma_start(out=t[:], in_=x_in[:, s * R:s * R + R + 2, :])
            xs_tiles[s] = t
            # +1 junk-guard row (see a3p); zeroed so reads stay finite.
            t8 = pa_x8.tile([128, R + 3, WP], F8, tag="xs8")
            nc.gpsimd.dma_start(out=t8[:, 0:R + 2, :],
                                in_=x8_in[:, s * R:s * R + R + 2, :])
            nc.vector.memset(t8[:, R + 2, :], 0.0)
            xs8_tiles[s] = t8

        load_xs(0)
        nc.sync.dma_start(out=w2t[:], in_=w2_in[:])
        nc.sync.dma_start(out=w3t8[:], in_=w3_in[:])
        # ident/gsum/mask4 are not needed until softmax/BN; issue after xs(0)
        nc.sync.dma_start(out=ident[:], in_=id_in[:])
        nc.sync.dma_start(out=gsum[:], in_=gp_in[:])
        nc.sync.dma_start(out=mask4[:], in_=mk_in[:])
        # warm-up collective at kernel start: absorbs the CC engine's ~11us
        # cold-start and any cross-core launch skew during pass A, so the
        # real AllReduce sees a warm mesh.  Input is the gsum const.
        cc_pool = tc.alloc_tile_pool(name="ccd", bufs=1, space="DRAM")
        cc_in1 = cc_pool.tile([128, 16], F32, tag="cc_in1")
        cc_out1 = cc_pool.tile([128, 16], F32, tag="cc_out1")
        cc_inb = cc_pool.tile([128, 1], F32, tag="cc_inb")
        cc_outb = cc_pool.tile([128, 1], F32, tag="cc_outb")
        nc.gpsimd.dma_start(out=cc_inb[:], in_=gsum[:, 0:1])
        nc.gpsimd.collective_compute(
            "AllReduce", mybir.AluOpType.add,
            replica_groups=[list(range(n_cores))],
            ins=[cc_inb.opt()], outs=[cc_outb.opt()])
        nc.gpsimd.dma_start(out=scalars[:, 15:16], in_=cc_outb[:])
        def emit_a3_conv(s):
            """fp8 DoubleRow a3 conv for strip s (4 pairs + 1 single per tile).

            rhs streams a flat 2*WP window: out row 0 lands in PSUM cols
            [0,W), row 1 in [WP, WP+W); 4 junk cols are skipped.
            """
            y0 = s * R
            xs8 = xs8_tiles.pop(s)
            for t in range(NT):
                cps = pa_cps8.tile([128, 2 * WP], F32, tag="cps8")
                for p, (dy, dx, dlt) in enumerate(_dr_pairs(WP)):
                    rhs = _rap(xs8[:], [[dlt, 2], [1, 2 * WP]],
                               (2 * t + dy) * WP + dx)
                    nc.tensor.matmul(cps[:, :], w3t8[:, 2 * p:2 * p + 2, :],
                                     rhs, start=(p == 0), stop=False,
                                     perf_mode=DR)
                rhs = _rap(xs8[:], [[1, 2 * WP]], (2 * t + 2) * WP + 2)
                nc.tensor.matmul(cps[:, :], w3t8[:, 8, :], rhs,
                                 start=False, stop=True)
                nc.scalar.mul(
                    a3p[:, 1 + y0 + 2 * t:1 + y0 + 2 * t + 2, 1:1 + W],
                    _rap(cps[:], [[WP, 2], [1, W]]), 1.0 / W3S)

        for s in range(NS):
            if s + 1 < NS:
                load_xs(s + 1)   # issue before this strip's dependent DMAs
            xs = xs_tiles[s]

            a1g = pa_g.tile([128, 9, P], BF16, tag="a1g")
            a2g = pa_g.tile([128, 9, P], BF16, tag="a2g")
            for t in range(NT):
                # -------- a1/a2: bf16, 9 offset matmuls ----------------------
                for wt, gdst in ((w1t, a1g), (w2t, a2g)):
                    cps = pa_cps.tile([128, 2 * W], F32, tag="cps")
                    for k in range(9):
                        dy, dx = divmod(k, 3)
                        rhs = xs[:, 2 * t + dy:2 * t + dy + 2, dx:dx + W]
                        nc.tensor.matmul(cps[:, :], wt[:, k, :], rhs,
                                         start=(k == 0), stop=(k == 8))
                    # scatter rows (2t, 2t+1) into subgrid-major layout
                    ya, yb = 2 * t, 2 * t + 1
                    ha, ra = ya % 3, ya // 3
                    hb, rb = yb % 3, yb // 3
                    offa = (3 * ha) * P + ra * Wq
                    sd = (3 * hb) * P + rb * Wq - offa
                    src = _rap(cps[:], [[W, 2], [1, 3], [3, Wq]])
                    dst = _rap(gdst[:], [[sd, 2], [P, 3], [1, Wq]], offa)
                    if gdst is a1g:
                        nc.vector.tensor_copy(dst, src)
                    else:
                        nc.scalar.copy(out=dst, in_=src)
                # the last 2 strips' a3 convs are deferred past the final
                # a1/a2 strip: their PE time covers the last strip's XBAR
                # transposes, which otherwise leave the PE idle.
            if s < NS - 2:
                emit_a3_conv(s)
            # XBAR transposes for this strip (run during next strip's convs)
            a1T = []
            a2T = []
            for hw in range(9):
                t1 = pa_t.tile([128, NE, 128], BF16, tag="a1T")
                nc.sync.dma_start_transpose(out=t1[:], in_=a1g[:, hw, :])
                a1T.append(t1)
                t2 = pa_t.tile([128, NE, 128], BF16, tag="a2T")
                nc.sync.dma_start_transpose(out=t2[:], in_=a2g[:, hw, :])
                a2T.append(t2)
            aT_tiles[s] = (a1T, a2T)
            if s >= 1:
                emit_attn_mms(s - 1)
        emit_a3_conv(NS - 2)
        emit_a3_conv(NS - 1)
        emit_attn_mms(NS - 1)

        pa_cps8.release(); pa_cps.release()
        pa_t.release(); pa_g.release(); pa_x8.release()

        # ---------------- softmax + kern transposes -------------------------
        if level >= 2:
            sm_pool = tc.alloc_tile_pool(name="smx", bufs=1)
            ssum = scalars[:, 2:3]
            rsum = scalars[:, 3:4]
            rs128 = scalars[:, 14:15]
            # logits*SCL are bounded (|attn| < ~10 sigma -> exp < e^19), so
            # the max-subtraction is unnecessary; exp reads PSUM directly.
            esb = sm_pool.tile([128, 9 * 128], F32, tag="esb")
            nc.scalar.activation(esb[:], attn_ps[:],
                                 mybir.ActivationFunctionType.Exp,
                                 scale=SCL)
            attn_psp.release()
            nc.vector.reduce_sum(ssum, esb[:], axis=mybir.AxisListType.X)
            nc.vector.reciprocal(rsum, ssum)
            nc.vector.tensor_scalar_mul(rs128, rsum, float(KSCL))
            sm_bf = sm_pool.tile([128, 9 * 128], BF16, tag="sm_bf")
            nc.vector.tensor_scalar_mul(sm_bf[:], esb[:], rs128)
            k_tps = tc.alloc_tile_pool(name="k_tps", bufs=3, space="PSUM")
            for hw in range(9):
                tp = k_tps.tile([128, 128], BF16, tag="ktp")
                nc.tensor.transpose(tp[:], sm_bf[:, hw * 128:(hw + 1) * 128],
                                    ident[:])
                nc.vector.tensor_copy(kernT[:, hw, :], tp[:])
            k_tps.release()
            sm_pool.release()
        else:
            attn_psp.release()

        # ---------------- pass B: dynamic conv + stats + permuted store -----
        # pass C strips in avp-availability order: out strip t (and t+4) is
        # fully written once pass B strip 2t+1 is scattered.
        PC_ORDER = [t for pair in zip(range(NS // 2), range(NS // 2, NS))
                    for t in pair]
        if level >= 3:
            pb_av = tc.alloc_tile_pool(name="pb_av", bufs=4)
            pb_sq = tc.alloc_tile_pool(name="pb_sq", bufs=1)
            pb_cps = tc.alloc_tile_pool(name="pb_cps", bufs=7, space="PSUM")
            for s in range(NS):
                if s == NS - 1:
                    # AR payload piece A (strips 0..NS-2): reduce + DMA to
                    # cc_in1 now, hidden under the last strip's compute; the
                    # trigger then only waits for piece B (strip NS-1's raw
                    # stats, DMA'd without a reduce).  Summed post-AR.
                    nc.vector.reduce_sum(
                        _rap(sglob[:], [[2, 4], [1, 2]]),
                        _rap(stats_cols[:], [[2, 4], [1, 2], [8, NS - 1]]),
                        axis=mybir.AxisListType.X)
                    nc.scalar.dma_start(out=cc_in1[:, 0:8], in_=sglob[:])
                y0 = s * R
                # av parity-split: av_sp[c, 2i+j, t, q] = av[c, 2t+i, 2q+j]
                av_sp = pb_av.tile([128, 4, NT, Wh], F8, tag="av")
                for t in range(NT):
                    cps = pb_cps.tile([128, 2 * WP], F32, tag="cps2")
                    for p, (dy, dx, dlt) in enumerate(_dr_pairs(WP)):
                        rhs = _rap(a3p[:], [[dlt, 2], [1, 2 * WP]],
                                   (y0 + 2 * t + dy) * WP + dx)
                        nc.tensor.matmul(cps[:, :],
                                         kernT[:, 2 * p:2 * p + 2, :],
                                         rhs, start=(p == 0), stop=False,
                                         perf_mode=DR)
                    rhs = _rap(a3p[:], [[1, 2 * WP]],
                               (y0 + 2 * t + 2) * WP + 2)
                    nc.tensor.matmul(cps[:, :], kernT[:, 8, :], rhs,
                                     start=False, stop=True)
                    # parity split out of the plain row-major PSUM window
                    src = _rap(cps[:], [[WP, 2], [1, 2], [2, Wh]])
                    dst = _rap(av_sp[:], [[2 * PQ, 2], [PQ, 2], [1, Wh]],
                               t * Wh)
                    if t % 2 == 0:
                        nc.vector.tensor_scalar_mul(dst, src, 1.0 / KSCL)
                    else:
                        nc.scalar.mul(dst, src, 1.0 / KSCL)
                sq = pb_sq.tile([128, PQ], F8, tag="sq")
                for pi in range(4):
                    psrc = _rap(av_sp[:], [[1, PQ]], pi * PQ)
                    nc.vector.reduce_sum(stats_cols[:, s, pi, 0:1], psrc,
                                         axis=mybir.AxisListType.X)
                    nc.scalar.activation(
                        out=sq[:], in_=psrc,
                        func=mybir.ActivationFunctionType.Square,
                        accum_out=stats_cols[:, s, pi, 1:2])
                # partition-crossing half of feature_map_stack:
                # avq[4*c1+pi, (s*4+c2)*PQ + n] = av_sp[32*c2+c1, pi*PQ + n]
                for pi in range(4):
                    for c2 in range(4):
                        qsrc = _rap(av_sp[32 * c2:32 * (c2 + 1)],
                                    [[1, PQ]], pi * PQ)
                        dst = bass.AP(
                            tensor=avq_d.tensor,
                            offset=(avq_d.offset + pi * H * W
                                    + (s * 4 + c2) * PQ),
                            ap=[[4 * H * W, 32], [1, PQ]])
                        nc.sync.dma_start(out=dst, in_=qsrc)
            pb_cps.release()
            pb_sq.release(); pb_av.release()

        # ---------------- AllReduce + BN coefficients -----------------------
        if level >= 4:
            nc.scalar.dma_start(out=cc_in1[:, 8:16],
                                in_=_rap(stats_cols[:], [[1, 8]], (NS - 1) * 8))
            nc.gpsimd.collective_compute(
                "AllReduce", mybir.AluOpType.add,
                replica_groups=[list(range(n_cores))],
                ins=[cc_in1.opt()], outs=[cc_out1.opt()])
            nc.scalar.dma_start(out=sglob2[:], in_=cc_out1[:])
            nc.vector.tensor_add(sglob_g[:], sglob2[:, 0:8], sglob2[:, 8:16])

            # a3p/kern are dead; free them so pass C gets deep buffer pools,
            # and issue every avp readback DMA now — they run under the AR.
            a3_pool.release()
            kern_pool.release()
            if level >= 5:
                # out strip t covers quadrant row c2hi=t//4, source strips
                # s0=2*(t%4) and s0+1; each half is one contiguous 2*PQ read.
                pc_a = tc.alloc_tile_pool(name="pc_a", bufs=8)
                pca_tiles = {}
                for t in PC_ORDER:
                    s0, c2hi = 2 * (t % 4), t // 4
                    avr = pc_a.tile([128, 2, 2, NT, Wh], F8, tag="av_s")
                    for sl in (0, 1):
                        off = ((s0 + sl) * 4 + 2 * c2hi) * PQ
                        nc.sync.dma_start(out=avr[:, sl],
                                          in_=avq_d[:, off:off + 2 * PQ])
                    pca_tiles[t] = avr

            # ------------ BN coefficients (per out-channel) -----------------
            bn_ps = tc.alloc_tile_pool(name="bn_ps", bufs=1, space="PSUM")
            gps = bn_ps.tile([128, 8], F32, tag="gps")
            nc.tensor.matmul(gps[:], gsum[:], sglob_g[:], start=True, stop=True)
            nc.vector.tensor_copy(msb[:], gps[:])
            bn_ps.release()
            mean = scalars[:, 4:5]
            e2 = scalars[:, 5:6]
            msq = scalars[:, 6:7]
            var = scalars[:, 7:8]
            sd = scalars[:, 8:9]
            rstd = scalars[:, 9:10]
            sc = scalars[:, 10:11]
            bb0 = scalars[:, 11:12]
            bb = scalars[:, 12:13]
            nc.vector.tensor_mul(sel[:], _rap(msb[:], [[2, 4]]), mask4[:])
            nc.vector.reduce_sum(mean, sel[:], axis=mybir.AxisListType.X)
            nc.vector.tensor_scalar_mul(mean, mean, 1.0 / N_TOT)
            nc.vector.tensor_mul(sel[:], _rap(msb[:], [[2, 4]], 1), mask4[:])
            nc.vector.reduce_sum(e2, sel[:], axis=mybir.AxisListType.X)
            nc.vector.tensor_scalar_mul(e2, e2, 1.0 / N_TOT)
            nc.vector.tensor_mul(msq, mean, mean)
            nc.vector.tensor_tensor(out=var, in0=e2, in1=msq,
                                    op=mybir.AluOpType.subtract)
            eps_ap = scalars[:, 13:14]
            nc.vector.memset(eps_ap, EPS)
            nc.scalar.activation(sd, var, mybir.ActivationFunctionType.Sqrt,
                                 bias=eps_ap)
            nc.vector.reciprocal(rstd, sd)
            nc.vector.tensor_scalar_mul(sc, rstd, NORM_SCALE)
            nc.vector.tensor_mul(bb0, mean, sc)
            nc.vector.tensor_scalar_mul(bb, bb0, -1.0)

        # ---------------- pass C: out = cm*x + sc*avp + bb ------------------
        if level >= 5:
            pc_t = tc.alloc_tile_pool(name="pc_t", bufs=3)
            pc_o = tc.alloc_tile_pool(name="pc_o", bufs=3)
            for idx, t in enumerate(PC_ORDER):
                y0 = t * R
                x_sv = xs_tiles[t][:, 1:1 + R, 1:1 + W]
                if cm != 1.0:
                    xc = pc_t.tile([128, R, W], BF16, tag="xc")
                    nc.scalar.mul(xc[:], x_sv, float(cm))
                    x_sv = xc[:]
                avr = pca_tiles.pop(t)
                # t = sc*av + bb per half strip (the op's APs also undo the
                # avq spatial interleave: row r = sl*NT+t', col X = c2lo*Wh+q),
                # then out = t + x on DVE.  Act takes 12 of the 16 halves
                # (DVE's 8 adds make it the binding engine otherwise).
                # NB: gpsimd must stay idle here -- gpsimd SBUF ops lock DVE
                # out of 2x-port mode and double every DVE op's latency.
                t_s = pc_t.tile([128, R, W], BF16, tag="t_s")
                o_s = pc_o.tile([128, R, W], BF16, tag="o_s")
                for sl in (0, 1):
                    tdst = _rap(t_s[:], [[W, NT], [Wh, 2], [1, Wh]],
                                sl * NT * W)
                    tsrc = _rap(avr[:], [[Wh, NT], [PQ, 2], [1, Wh]],
                                sl * 2 * PQ)
                    if sl == 0 or idx % 2 == 0:
                        nc.scalar.activation(tdst, tsrc,
                                             mybir.ActivationFunctionType.Identity,
                                             bias=bb, scale=sc)
                    else:
                        nc.vector.tensor_scalar(out=tdst, in0=tsrc,
                                                scalar1=sc, scalar2=bb,
                                                op0=mybir.AluOpType.mult,
                                                op1=mybir.AluOpType.add)
                nc.vector.tensor_add(o_s[:], t_s[:], x_sv)
                nc.sync.dma_start(out=out_d[:, y0:y0 + R, :], in_=o_s[:])
            pc_o.release(); pc_t.release()

        if level >= 5:
            pc_a.release()
        cc_pool.release()
        pa_x.release()
        small.release()
        consts.release()

    nc.compile()
    return nc


def _prep_wt(w, permute_out=False):
    """[Co,Ci,3,3] -> lhsT layout [Ci, 9, Co] (optionally out-chan permuted)."""
    import ml_dtypes
    wt = np.ascontiguousarray(w.transpose(1, 2, 3, 0).reshape(128, 9, 128))
    if permute_out:
        p = np.arange(128)
        co_of_p = 4 * (p % 32) + p // 32     # partition p holds channel co_of_p
        wt = np.ascontiguousarray(wt[:, :, co_of_p])
    return wt


def make_const_inputs(w1, w2, w3):
    import ml_dtypes
    p = np.arange(128)
    # gsum[p_src, C']: sum av partitions with p_src%32 == C'//4
    gsum = (p[:, None] % 32 == p[None, :] // 4).astype(np.float32)
    mask4 = (p[:, None] % 4 == np.arange(4)[None, :]).astype(np.float32)
    w3t = _prep_wt(np.asarray(w3, np.float32)) * W3S
    return {
        "ident": np.eye(128, dtype=np.float32).astype(ml_dtypes.bfloat16),
        "w1t": _prep_wt(np.asarray(w1, np.float32)).astype(ml_dtypes.bfloat16),
        "w2t": _prep_wt(np.asarray(w2, np.float32), permute_out=True)
        .astype(ml_dtypes.bfloat16),
        "w3t8": np.clip(w3t, -240, 240).astype(ml_dtypes.float8_e4m3),
        "gsum": gsum,
        "mask4": mask4,
    }


def pad_x(x_sample):
    import ml_dtypes
    return np.pad(x_sample, ((0, 0), (1, 1), (1, 1))).astype(ml_dtypes.bfloat16)


def pad_x8(x_sample):
    import ml_dtypes
    return (np.clip(np.pad(x_sample, ((0, 0), (1, 1), (1, 1))), -240, 240)
            .astype(ml_dtypes.float8_e4m3))


_CACHE = {}


def kernel(x, w1, w2, w3, conv_momentum):
    from concourse.bass_utils import run_bass_kernel_spmd

    x = np.asarray(x, np.float32)
    B, Ci, H, W = x.shape
    cm = float(np.asarray(conv_momentum))
    key = (H, W, B, cm)
    if key not in _CACHE:
        _CACHE[key] = build_nc(H, W, 24, B, cm)
    nc = _CACHE[key]
    consts = make_const_inputs(w1, w2, w3)
    in_maps = [dict(consts, x=pad_x(x[b]), x8=pad_x8(x[b])) for b in range(B)]
    res = run_bass_kernel_spmd(nc, in_maps, list(range(B)))
    out = np.stack(
        [np.asarray(res.results[b]["out"]).reshape(128, H, W) for b in range(B)],
        axis=0)
    return out.astype(np.float32)


# revision 35
# speedup vs baseline: 1.0206x; 1.0206x over previous
"""Trainium2 Bass kernel for nn_AttnConv2d (attention-conv + dynamic conv + BN).

Math (per sample b):
  a1 = conv3x3(x, w1); a2 = conv3x3(x, w2); a3 = conv3x3(x, w3)     (SAME pad)
  attn[h,w,i,o] = sum_{p,q} a1[i,3p+h,3q+w] * a2[o,3p+h,3q+w]
  kern[o,:,:,:] = softmax(attn[.,.,.,o] / sqrt(Ci*9))
  av = conv3x3(a3, kern[b])                                         (per-sample kernel)
  y  = feature_map_stack(av)   (pure spatial/channel permutation)
  out = cm * x + NORM_SCALE * (y - mean_y) * rsqrt(var_y + eps)     (batch stats)

Sharding: data-parallel over batch, 1 sample per core, 8 cores.  The only
cross-core exchange is an AllReduce of the per-channel BN partial sums; a
warm-up collective fires early so the real one pays no setup cost.

Implementation notes:
  - a1/a2 convs run bf16 on the PE (f32 PSUM accumulate); the a3 conv and the
    dynamic per-sample conv run fp8e4 with perf_mode=DoubleRow: kernel-offset
    pairs are fed as the DR contraction pair (rhs pair stride = the spatial
    delta between the two offsets), so 9 offset matmuls become 4 DR + 1
    normal.  w3 is host-scaled x16 (undone in the PSUM->a3p copy), softmax
    output is scaled x128 into fp8 (undone in the PSUM->av copy); a3 is
    stored fp8 so the dynamic conv rhs reads it directly.
  - the DoubleRow rhs must be [K, 2, N] (one flat N dim), so conv rows are
    streamed as one flat 2*WP window: out row 0 lands in PSUM cols [0,W),
    row 1 at [WP,WP+W), 4 junk cols are skipped by the PSUM copy (xs8/a3p
    carry a zeroed junk-guard row for the last tile's overrun).
  - attention needs positions on the partition axis: a1g/a2g strips are
    transposed by the DMA XBAR (dma_start_transpose) instead of the PE; attn
    matmuls for strip s issue after the convs of strip s+1.  The LAST two
    strips' a3 convs are deferred past the final a1/a2 strip so their PE time
    covers the last strip's XBAR transposes.
  - av goes to DRAM through a channel-planar scratch avq[co, s, c2, t', q]:
    the partition-crossing half of feature_map_stack happens in the scatter
    (32 contiguous fp8 PQ-runs per DMA -- NOT the output layout, whose 192B
    runs cost ~50k DMA descriptors and starve the AllReduce's fabric packets
    of DMA-engine slots); the spatial (Y,X) interleave is undone for free by
    the pass-C engine APs.  The scratch is fp8 (quantization is inside the
    BN, so it costs ~1e-3 rel err and halves scatter/readback bytes).
  - x strips loaded for pass A are RETAINED in SBUF (pa_x bufs=NS) and reused
    as the pass-C residual, so pass C issues no x reloads at all.
  - a warm-up AllReduce fires at kernel START (input: a const tile): the CC
    engine's ~11us mesh cold-start and any cross-core launch skew get paid
    under pass A.  A warm-up right before the real AR is poison: collectives
    serialize on the CC core, so a late-arriving warm-up delays the real one.
  - the AllReduce payload is split: strips 0..NS-2 partial sums are reduced
    and DMA'd to the collective input under the last strip's compute; the
    trigger then only waits for strip NS-1's raw stats columns (no reduce),
    and the two halves are summed post-AR.
  - after pass B, a3p/kern are freed and the avq readback pool is allocated;
    readback DMAs issue under the AllReduce window.  Pass C applies
    t = sc*av + bb per half strip (Act Identity for 11 of 16 halves, DVE
    tensor_scalar for 5), then DVE adds the residual.  GpSimd must stay idle
    here: gpsimd SBUF ops lock DVE out of 2x-port mode.
"""

import os
import sys

for _p in ("/opt/trn_rl_repo", "/root/.axon_site/_ro/trn_rl_repo"):
    if os.path.isdir(_p) and _p not in sys.path:
        sys.path.insert(0, _p)
        break

import numpy as np

import concourse.bass as bass
import concourse.bacc as bacc
import concourse.tile as tile
from concourse import mybir

F32 = mybir.dt.float32
BF16 = mybir.dt.bfloat16
F8 = mybir.dt.float8e4
DR = mybir.MatmulPerfMode.DoubleRow

ATTN_K = 3
NH = 2
EPS = 1e-5
NORM_SCALE = 0.1816
CI = 128
CO = 128
W3S = 16.0     # host scale on w3 (fp8), undone in the PSUM->a3p copy
KSCL = 128.0   # scale on softmax output (fp8), undone in the PSUM->av copy

# DoubleRow offset pairing: (dy, dx, rhs pair stride as fn of WP)
# pairs are (k0,k1) (k2,k3) (k4,k5) (k6,k7) of the row-major 3x3 offsets,
# matching weight layout [128, 9, 128] sliced [:, 2p:2p+2, :]; k8 is single.
def _dr_pairs(WP):
    return [(0, 0, 1), (0, 2, WP - 2), (1, 1, 1), (2, 0, 1)]


def _rap(base, dims, off=0):
    """Raw AP on the same tensor as `base` (keeps base's partition dim)."""
    return bass.AP(tensor=base.tensor, offset=base.offset + off,
                   ap=[base.ap[0]] + [list(d) for d in dims])


def build_nc(H, W, R, n_cores, cm, level=5, pb_parity=True):
    """Build the per-core Bass kernel. R = strip rows (div by 6, even)."""
    assert H % R == 0 and R % 6 == 0 and W % 6 == 0
    NS = H // R                      # strips
    Wq = W // 3                      # attn subgrid cols
    P = (R // 3) * Wq                # attn positions per offset per strip
    S = H // 2                       # quadrant size of feature_map_stack
    NT = R // 2                      # psum tiles (2 rows) per strip
    Wh = W // 2
    PQ = NT * Wh                     # parity-split positions per strip
    N_TOT = float(n_cores * H * W)   # BN count per channel
    SCL = 1.0 / float(np.sqrt(CI * 9))
    WP = W + 2                       # padded row length
    assert P % 128 == 0
    NE = P // 128                    # 128-position chunks per offset per strip

    nc = bacc.Bacc("TRN2", target_bir_lowering=False, debug=False,
                   num_devices=n_cores)

    x_in = nc.dram_tensor("x", [128, H + 2, WP], BF16,
                          kind="ExternalInput").ap()   # host-padded (+1 ring)
    x8_in = nc.dram_tensor("x8", [128, H + 2, WP], F8,
                           kind="ExternalInput").ap()  # fp8 copy of x
    w1_in = nc.dram_tensor("w1t", [128, 9, 128], BF16, kind="ExternalInput").ap()
    w2_in = nc.dram_tensor("w2t", [128, 9, 128], BF16, kind="ExternalInput").ap()
    w3_in = nc.dram_tensor("w3t8", [128, 9, 128], F8, kind="ExternalInput").ap()
    id_in = nc.dram_tensor("ident", [128, 128], BF16, kind="ExternalInput").ap()
    gp_in = nc.dram_tensor("gsum", [128, 128], F32, kind="ExternalInput").ap()
    mk_in = nc.dram_tensor("mask4", [128, 4], F32, kind="ExternalInput").ap()
    out_d = nc.dram_tensor("out", [128, H, W], BF16, kind="ExternalOutput").ap()
    # channel-planar scratch: avq[co, s, c2, t', q] -- the partition-crossing
    # half of feature_map_stack happens in the scatter (32 contiguous PQ-runs
    # per DMA), the spatial (Y,X) reshuffle happens in pass-C engine APs.
    avq_d = nc.dram_tensor("avq", [128, H * W], F8).ap()

    with tile.TileContext(nc) as tc:
        consts = tc.alloc_tile_pool(name="consts", bufs=1)
        w1t = consts.tile([128, 9, 128], BF16, tag="w1t")
        w2t = consts.tile([128, 9, 128], BF16, tag="w2t")
        w3t8 = consts.tile([128, 9, 128], F8, tag="w3t8")
        ident = consts.tile([128, 128], BF16, tag="ident")
        gsum = consts.tile([128, 128], F32, tag="gsum")
        mask4 = consts.tile([128, 4], F32, tag="mask4")
        nc.sync.dma_start(out=w1t[:], in_=w1_in[:])

        small = tc.alloc_tile_pool(name="small", bufs=1)
        stats_cols = small.tile([128, NS, 4, 2], F32, tag="stats_cols")
        sglob = small.tile([128, 8], F32, tag="sglob")
        sglob_g = small.tile([128, 8], F32, tag="sglob_g")
        sglob2 = small.tile([128, 16], F32, tag="sglob2")
        scalars = small.tile([128, 16], F32, tag="scalars")
        msb = small.tile([128, 8], F32, tag="msb")
        sel = small.tile([128, 4], F32, tag="sel")

        # pa_x sits low in the pool stack: its strips are retained through
        # pass C (the residual), while kern/a3p above it release after pass B.
        pa_x = tc.alloc_tile_pool(name="pa_x", bufs=NS)

        kern_pool = tc.alloc_tile_pool(name="kern", bufs=1)
        kernT = kern_pool.tile([128, 9, 128], F8, tag="kT", name="kT")

        # +1 junk-guard row: the flat 2*WP-column DoubleRow rhs of the last
        # tile reads 2 elements past row H+1 (results land in skipped cols).
        a3_pool = tc.alloc_tile_pool(name="a3p", bufs=1)
        a3p = a3_pool.tile([128, H + 3, WP], F8, tag="a3p")
        # zero the pad border of a3p once
        nc.vector.memset(_rap(a3p[:], [[1, WP]]), 0.0)                      # row 0
        nc.vector.memset(_rap(a3p[:], [[1, 2 * WP]], (H + 1) * WP), 0.0)    # rows H+1,H+2
        nc.vector.memset(_rap(a3p[:], [[WP, H + 3]]), 0.0)                  # col 0
        nc.vector.memset(_rap(a3p[:], [[WP, H + 3]], W + 1), 0.0)           # col W+1

        attn_psp = tc.alloc_tile_pool(name="attn_ps", bufs=1, space="PSUM")
        attn_ps = attn_psp.tile([128, 9 * 128], F32, tag="attn")

        # ---------------- pass A: static convs + attention accumulation ------
        pa_x8 = tc.alloc_tile_pool(name="pa_x8", bufs=2)
        pa_g = tc.alloc_tile_pool(name="pa_g", bufs=2)
        # 2 tags x 18 bufs: tile (s,hw) recycles (s-2,hw)'s buffer, whose attn
        # MMs were emitted before strip s-1's transposes -- no stall.
        pa_t = tc.alloc_tile_pool(name="pa_t", bufs=18)
        pa_cps = tc.alloc_tile_pool(name="pa_cps", bufs=3, space="PSUM")
        pa_cps8 = tc.alloc_tile_pool(name="pa_cps8", bufs=2, space="PSUM")

        aT_tiles = {}

        def emit_attn_mms(s):
            a1T, a2T = aT_tiles.pop(s)
            for hw in range(9):
                for e in range(NE):
                    nc.tensor.matmul(
                        attn_ps[:, hw * 128:(hw + 1) * 128],
                        a2T[hw][:, e, :], a1T[hw][:, e, :],
                        start=(s == 0 and e == 0 and hw in (0, 4, 8)),
                        stop=(s == NS - 1 and e == NE - 1 and hw in (3, 7, 8)),
                        skip_group_check=True)

        xs_tiles = {}
        xs8_tiles = {}

        def load_xs(s):
            t = pa_x.tile([128, R + 2, WP], BF16, tag="xs")
            if s == 0:
                # split the first strip so tile-0 matmuls start ~3us earlier
                nc.sync.dma_start(out=t[:, 0:4, :], in_=x_in[:, 0:4, :])
                nc.sync.dma_start(out=t[:, 4:14, :], in_=x_in[:, 4:14, :])
                nc.sync.dma_start(out=t[:, 14:R + 2, :],
                                  in_=x_in[:, 14:R + 2, :])
            else:
                nc.sync.dma_start(out=t[:], in_=x_in[:, s * R:s * R + R + 2, :])
            xs_tiles[s] = t
            # +1 junk-guard row (see a3p); zeroed so reads stay finite.
            t8 = pa_x8.tile([128, R + 3, WP], F8, tag="xs8")
            nc.gpsimd.dma_start(out=t8[:, 0:R + 2, :],
                                in_=x8_in[:, s * R:s * R + R + 2, :])
            nc.vector.memset(t8[:, R + 2, :], 0.0)
            xs8_tiles[s] = t8

        load_xs(0)
        nc.sync.dma_start(out=w2t[:], in_=w2_in[:])
        nc.sync.dma_start(out=w3t8[:], in_=w3_in[:])
        # ident/gsum/mask4 are not needed until softmax/BN; issue after xs(0)
        nc.sync.dma_start(out=ident[:], in_=id_in[:])
        nc.sync.dma_start(out=gsum[:], in_=gp_in[:])
        nc.sync.dma_start(out=mask4[:], in_=mk_in[:])
        # warm-up collective at kernel start: absorbs the CC engine's ~11us
        # cold-start and any cross-core launch skew during pass A, so the
        # real AllReduce sees a warm mesh.  Input is the gsum const.
        cc_pool = tc.alloc_tile_pool(name="ccd", bufs=1, space="DRAM")
        cc_in1 = cc_pool.tile([128, 16], F32, tag="cc_in1")
        cc_out1 = cc_pool.tile([128, 16], F32, tag="cc_out1")
        cc_inb = cc_pool.tile([128, 1], F32, tag="cc_inb")
        cc_outb = cc_pool.tile([128, 1], F32, tag="cc_outb")
        nc.gpsimd.dma_start(out=cc_inb[:], in_=gsum[:, 0:1])
        nc.gpsimd.collective_compute(
            "AllReduce", mybir.AluOpType.add,
            replica_groups=[list(range(n_cores))],
            ins=[cc_inb.opt()], outs=[cc_outb.opt()])
        nc.gpsimd.dma_start(out=scalars[:, 15:16], in_=cc_outb[:])
        def emit_a3_conv(s):
            """fp8 DoubleRow a3 conv for strip s (4 pairs + 1 single per tile).

            rhs streams a flat 2*WP window: out row 0 lands in PSUM cols
            [0,W), row 1 in [WP, WP+W); 4 junk cols are skipped.
            """
            y0 = s * R
            xs8 = xs8_tiles.pop(s)
            for t in range(NT):
                cps = pa_cps8.tile([128, 2 * WP], F32, tag="cps8")
                for p, (dy, dx, dlt) in enumerate(_dr_pairs(WP)):
                    rhs = _rap(xs8[:], [[dlt, 2], [1, 2 * WP]],
                               (2 * t + dy) * WP + dx)
                    nc.tensor.matmul(cps[:, :], w3t8[:, 2 * p:2 * p + 2, :],
                                     rhs, start=(p == 0), stop=False,
                                     perf_mode=DR)
                rhs = _rap(xs8[:], [[1, 2 * WP]], (2 * t + 2) * WP + 2)
                nc.tensor.matmul(cps[:, :], w3t8[:, 8, :], rhs,
                                 start=False, stop=True)
                nc.scalar.mul(
                    a3p[:, 1 + y0 + 2 * t:1 + y0 + 2 * t + 2, 1:1 + W],
                    _rap(cps[:], [[WP, 2], [1, W]]), 1.0 / W3S)

        for s in range(NS):
            if s + 1 < NS:
                load_xs(s + 1)   # issue before this strip's dependent DMAs
            xs = xs_tiles[s]

            a1g = pa_g.tile([128, 9, P], BF16, tag="a1g")
            a2g = pa_g.tile([128, 9, P], BF16, tag="a2g")
            for t in range(NT):
                # -------- a1/a2: bf16, 9 offset matmuls ----------------------
                for wt, gdst in ((w1t, a1g), (w2t, a2g)):
                    cps = pa_cps.tile([128, 2 * W], F32, tag="cps")
                    for k in range(9):
                        dy, dx = divmod(k, 3)
                        rhs = xs[:, 2 * t + dy:2 * t + dy + 2, dx:dx + W]
                        nc.tensor.matmul(cps[:, :], wt[:, k, :], rhs,
                                         start=(k == 0), stop=(k == 8))
                    # scatter rows (2t, 2t+1) into subgrid-major layout
                    ya, yb = 2 * t, 2 * t + 1
                    ha, ra = ya % 3, ya // 3
                    hb, rb = yb % 3, yb // 3
                    offa = (3 * ha) * P + ra * Wq
                    sd = (3 * hb) * P + rb * Wq - offa
                    src = _rap(cps[:], [[W, 2], [1, 3], [3, Wq]])
                    dst = _rap(gdst[:], [[sd, 2], [P, 3], [1, Wq]], offa)
                    if gdst is a1g:
                        nc.vector.tensor_copy(dst, src)
                    else:
                        nc.scalar.copy(out=dst, in_=src)
                # the last 2 strips' a3 convs are deferred past the final
                # a1/a2 strip: their PE time covers the last strip's XBAR
                # transposes, which otherwise leave the PE idle.
            if s < NS - 2:
                emit_a3_conv(s)
            # XBAR transposes for this strip (run during next strip's convs)
            a1T = []
            a2T = []
            for hw in range(9):
                t1 = pa_t.tile([128, NE, 128], BF16, tag="a1T")
                nc.sync.dma_start_transpose(out=t1[:], in_=a1g[:, hw, :])
                a1T.append(t1)
                t2 = pa_t.tile([128, NE, 128], BF16, tag="a2T")
                nc.sync.dma_start_transpose(out=t2[:], in_=a2g[:, hw, :])
                a2T.append(t2)
            aT_tiles[s] = (a1T, a2T)
            if s >= 1:
                emit_attn_mms(s - 1)
        emit_a3_conv(NS - 2)
        emit_a3_conv(NS - 1)
        emit_attn_mms(NS - 1)

        pa_cps8.release(); pa_cps.release()
        pa_t.release(); pa_g.release(); pa_x8.release()

        # ---------------- softmax + kern transposes -------------------------
        if level >= 2:
            sm_pool = tc.alloc_tile_pool(name="smx", bufs=1)
            ssum = scalars[:, 2:3]
            rsum = scalars[:, 3:4]
            rs128 = scalars[:, 14:15]
            # logits*SCL are bounded (|attn| < ~10 sigma -> exp < e^19), so
            # the max-subtraction is unnecessary; exp reads PSUM directly.
            esb = sm_pool.tile([128, 9 * 128], F32, tag="esb")
            nc.scalar.activation(esb[:], attn_ps[:],
                                 mybir.ActivationFunctionType.Exp,
                                 scale=SCL)
            attn_psp.release()
            nc.vector.reduce_sum(ssum, esb[:], axis=mybir.AxisListType.X)
            nc.vector.reciprocal(rsum, ssum)
            nc.vector.tensor_scalar_mul(rs128, rsum, float(KSCL))
            sm_bf = sm_pool.tile([128, 9 * 128], BF16, tag="sm_bf")
            nc.vector.tensor_scalar_mul(sm_bf[:], esb[:], rs128)
            k_tps = tc.alloc_tile_pool(name="k_tps", bufs=3, space="PSUM")
            for hw in range(9):
                tp = k_tps.tile([128, 128], BF16, tag="ktp")
                nc.tensor.transpose(tp[:], sm_bf[:, hw * 128:(hw + 1) * 128],
                                    ident[:])
                nc.vector.tensor_copy(kernT[:, hw, :], tp[:])
            k_tps.release()
            sm_pool.release()
        else:
            attn_psp.release()

        # ---------------- pass B: dynamic conv + stats + permuted store -----
        # pass C strips in avp-availability order: out strip t (and t+4) is
        # fully written once pass B strip 2t+1 is scattered.
        PC_ORDER = [t for pair in zip(range(NS // 2), range(NS // 2, NS))
                    for t in pair]
        if level >= 3:
            pb_av = tc.alloc_tile_pool(name="pb_av", bufs=4)
            pb_sq = tc.alloc_tile_pool(name="pb_sq", bufs=1)
            pb_cps = tc.alloc_tile_pool(name="pb_cps", bufs=7, space="PSUM")
            for s in range(NS):
                if s == NS - 1:
                    # AR payload piece A (strips 0..NS-2): reduce + DMA to
                    # cc_in1 now, hidden under the last strip's compute; the
                    # trigger then only waits for piece B (strip NS-1's raw
                    # stats, DMA'd without a reduce).  Summed post-AR.
                    nc.vector.reduce_sum(
                        _rap(sglob[:], [[2, 4], [1, 2]]),
                        _rap(stats_cols[:], [[2, 4], [1, 2], [8, NS - 1]]),
                        axis=mybir.AxisListType.X)
                    nc.scalar.dma_start(out=cc_in1[:, 0:8], in_=sglob[:])
                y0 = s * R
                # av parity-split: av_sp[c, 2i+j, t, q] = av[c, 2t+i, 2q+j]
                av_sp = pb_av.tile([128, 4, NT, Wh], F8, tag="av")
                for t in range(NT):
                    cps = pb_cps.tile([128, 2 * WP], F32, tag="cps2")
                    for p, (dy, dx, dlt) in enumerate(_dr_pairs(WP)):
                        rhs = _rap(a3p[:], [[dlt, 2], [1, 2 * WP]],
                                   (y0 + 2 * t + dy) * WP + dx)
                        nc.tensor.matmul(cps[:, :],
                                         kernT[:, 2 * p:2 * p + 2, :],
                                         rhs, start=(p == 0), stop=False,
                                         perf_mode=DR)
                    rhs = _rap(a3p[:], [[1, 2 * WP]],
                               (y0 + 2 * t + 2) * WP + 2)
                    nc.tensor.matmul(cps[:, :], kernT[:, 8, :], rhs,
                                     start=False, stop=True)
                    # parity split out of the plain row-major PSUM window
                    src = _rap(cps[:], [[WP, 2], [1, 2], [2, Wh]])
                    dst = _rap(av_sp[:], [[2 * PQ, 2], [PQ, 2], [1, Wh]],
                               t * Wh)
                    if t % 2 == 0:
                        nc.vector.tensor_scalar_mul(dst, src, 1.0 / KSCL)
                    else:
                        nc.scalar.mul(dst, src, 1.0 / KSCL)
                sq = pb_sq.tile([128, PQ], F8, tag="sq")
                for pi in range(4):
                    psrc = _rap(av_sp[:], [[1, PQ]], pi * PQ)
                    nc.vector.reduce_sum(stats_cols[:, s, pi, 0:1], psrc,
                                         axis=mybir.AxisListType.X)
                    nc.scalar.activation(
                        out=sq[:], in_=psrc,
                        func=mybir.ActivationFunctionType.Square,
                        accum_out=stats_cols[:, s, pi, 1:2])
                # partition-crossing half of feature_map_stack:
                # avq[4*c1+pi, (s*4+c2)*PQ + n] = av_sp[32*c2+c1, pi*PQ + n]
                for pi in range(4):
                    for c2 in range(4):
                        qsrc = _rap(av_sp[32 * c2:32 * (c2 + 1)],
                                    [[1, PQ]], pi * PQ)
                        dst = bass.AP(
                            tensor=avq_d.tensor,
                            offset=(avq_d.offset + pi * H * W
                                    + (s * 4 + c2) * PQ),
                            ap=[[4 * H * W, 32], [1, PQ]])
                        nc.sync.dma_start(out=dst, in_=qsrc)
            pb_cps.release()
            pb_sq.release(); pb_av.release()

        # ---------------- AllReduce + BN coefficients -----------------------
        if level >= 4:
            nc.scalar.dma_start(out=cc_in1[:, 8:16],
                                in_=_rap(stats_cols[:], [[1, 8]], (NS - 1) * 8))
            nc.gpsimd.collective_compute(
                "AllReduce", mybir.AluOpType.add,
                replica_groups=[list(range(n_cores))],
                ins=[cc_in1.opt()], outs=[cc_out1.opt()])
            nc.scalar.dma_start(out=sglob2[:], in_=cc_out1[:])
            nc.vector.tensor_add(sglob_g[:], sglob2[:, 0:8], sglob2[:, 8:16])

            # a3p/kern are dead; free them so pass C gets deep buffer pools,
            # and issue every avp readback DMA now — they run under the AR.
            a3_pool.release()
            kern_pool.release()
            if level >= 5:
                # out strip t covers quadrant row c2hi=t//4, source strips
                # s0=2*(t%4) and s0+1; each half is one contiguous 2*PQ read.
                pc_a = tc.alloc_tile_pool(name="pc_a", bufs=8)
                pca_tiles = {}
                for t in PC_ORDER:
                    s0, c2hi = 2 * (t % 4), t // 4
                    avr = pc_a.tile([128, 2, 2, NT, Wh], F8, tag="av_s")
                    for sl in (0, 1):
                        off = ((s0 + sl) * 4 + 2 * c2hi) * PQ
                        nc.sync.dma_start(out=avr[:, sl],
                                          in_=avq_d[:, off:off + 2 * PQ])
                    pca_tiles[t] = avr

            # ------------ BN coefficients (per out-channel) -----------------
            bn_ps = tc.alloc_tile_pool(name="bn_ps", bufs=1, space="PSUM")
            gps = bn_ps.tile([128, 8], F32, tag="gps")
            nc.tensor.matmul(gps[:], gsum[:], sglob_g[:], start=True, stop=True)
            nc.vector.tensor_copy(msb[:], gps[:])
            bn_ps.release()
            mean = scalars[:, 4:5]
            e2 = scalars[:, 5:6]
            msq = scalars[:, 6:7]
            var = scalars[:, 7:8]
            sd = scalars[:, 8:9]
            rstd = scalars[:, 9:10]
            sc = scalars[:, 10:11]
            bb0 = scalars[:, 11:12]
            bb = scalars[:, 12:13]
            nc.vector.tensor_mul(sel[:], _rap(msb[:], [[2, 4]]), mask4[:])
            nc.vector.reduce_sum(mean, sel[:], axis=mybir.AxisListType.X)
            nc.vector.tensor_scalar_mul(mean, mean, 1.0 / N_TOT)
            nc.vector.tensor_mul(sel[:], _rap(msb[:], [[2, 4]], 1), mask4[:])
            nc.vector.reduce_sum(e2, sel[:], axis=mybir.AxisListType.X)
            nc.vector.tensor_scalar_mul(e2, e2, 1.0 / N_TOT)
            nc.vector.tensor_mul(msq, mean, mean)
            nc.vector.tensor_tensor(out=var, in0=e2, in1=msq,
                                    op=mybir.AluOpType.subtract)
            eps_ap = scalars[:, 13:14]
            nc.vector.memset(eps_ap, EPS)
            nc.scalar.activation(sd, var, mybir.ActivationFunctionType.Sqrt,
                                 bias=eps_ap)
            nc.vector.reciprocal(rstd, sd)
            nc.vector.tensor_scalar_mul(sc, rstd, NORM_SCALE)
            nc.vector.tensor_mul(bb0, mean, sc)
            nc.vector.tensor_scalar_mul(bb, bb0, -1.0)

        # ---------------- pass C: out = cm*x + sc*avp + bb ------------------
        if level >= 5:
            pc_t = tc.alloc_tile_pool(name="pc_t", bufs=3)
            pc_o = tc.alloc_tile_pool(name="pc_o", bufs=3)
            for idx, t in enumerate(PC_ORDER):
                y0 = t * R
                x_sv = xs_tiles[t][:, 1:1 + R, 1:1 + W]
                if cm != 1.0:
                    xc = pc_t.tile([128, R, W], BF16, tag="xc")
                    nc.scalar.mul(xc[:], x_sv, float(cm))
                    x_sv = xc[:]
                avr = pca_tiles.pop(t)
                # t = sc*av + bb per half strip (the op's APs also undo the
                # avq spatial interleave: row r = sl*NT+t', col X = c2lo*Wh+q),
                # then out = t + x on DVE.  Act takes 11 of the 16 halves.
                # NB: gpsimd must stay idle here -- gpsimd SBUF ops lock DVE
                # out of 2x-port mode and double every DVE op's latency.
                t_s = pc_t.tile([128, R, W], BF16, tag="t_s")
                o_s = pc_o.tile([128, R, W], BF16, tag="o_s")
                for sl in (0, 1):
                    tdst = _rap(t_s[:], [[W, NT], [Wh, 2], [1, Wh]],
                                sl * NT * W)
                    tsrc = _rap(avr[:], [[Wh, NT], [PQ, 2], [1, Wh]],
                                sl * 2 * PQ)
                    if sl == 0 or idx % 3 == 0:
                        nc.scalar.activation(tdst, tsrc,
                                             mybir.ActivationFunctionType.Identity,
                                             bias=bb, scale=sc)
                    else:
                        nc.vector.tensor_scalar(out=tdst, in0=tsrc,
                                                scalar1=sc, scalar2=bb,
                                                op0=mybir.AluOpType.mult,
                                                op1=mybir.AluOpType.add)
                nc.vector.tensor_add(o_s[:], t_s[:], x_sv)
                nc.sync.dma_start(out=out_d[:, y0:y0 + R, :], in_=o_s[:])
            pc_o.release(); pc_t.release()

        if level >= 5:
            pc_a.release()
        cc_pool.release()
        pa_x.release()
        small.release()
        consts.release()

    nc.compile()
    return nc


def _prep_wt(w, permute_out=False):
    """[Co,Ci,3,3] -> lhsT layout [Ci, 9, Co] (optionally out-chan permuted)."""
    import ml_dtypes
    wt = np.ascontiguousarray(w.transpose(1, 2, 3, 0).reshape(128, 9, 128))
    if permute_out:
        p = np.arange(128)
        co_of_p = 4 * (p % 32) + p // 32     # partition p holds channel co_of_p
        wt = np.ascontiguousarray(wt[:, :, co_of_p])
    return wt


def make_const_inputs(w1, w2, w3):
    import ml_dtypes
    p = np.arange(128)
    # gsum[p_src, C']: sum av partitions with p_src%32 == C'//4
    gsum = (p[:, None] % 32 == p[None, :] // 4).astype(np.float32)
    mask4 = (p[:, None] % 4 == np.arange(4)[None, :]).astype(np.float32)
    w3t = _prep_wt(np.asarray(w3, np.float32)) * W3S
    return {
        "ident": np.eye(128, dtype=np.float32).astype(ml_dtypes.bfloat16),
        "w1t": _prep_wt(np.asarray(w1, np.float32)).astype(ml_dtypes.bfloat16),
        "w2t": _prep_wt(np.asarray(w2, np.float32), permute_out=True)
        .astype(ml_dtypes.bfloat16),
        "w3t8": np.clip(w3t, -240, 240).astype(ml_dtypes.float8_e4m3),
        "gsum": gsum,
        "mask4": mask4,
    }


def pad_x(x_sample):
    import ml_dtypes
    return np.pad(x_sample, ((0, 0), (1, 1), (1, 1))).astype(ml_dtypes.bfloat16)


def pad_x8(x_sample):
    import ml_dtypes
    return (np.clip(np.pad(x_sample, ((0, 0), (1, 1), (1, 1))), -240, 240)
            .astype(ml_dtypes.float8_e4m3))


_CACHE = {}


def kernel(x, w1, w2, w3, conv_momentum):
    from concourse.bass_utils import run_bass_kernel_spmd

    x = np.asarray(x, np.float32)
    B, Ci, H, W = x.shape
    cm = float(np.asarray(conv_momentum))
    key = (H, W, B, cm)
    if key not in _CACHE:
        _CACHE[key] = build_nc(H, W, 24, B, cm)
    nc = _CACHE[key]
    consts = make_const_inputs(w1, w2, w3)
    in_maps = [dict(consts, x=pad_x(x[b]), x8=pad_x8(x[b])) for b in range(B)]
    res = run_bass_kernel_spmd(nc, in_maps, list(range(B)))
    out = np.stack(
        [np.asarray(res.results[b]["out"]).reshape(128, H, W) for b in range(B)],
        axis=0)
    return out.astype(np.float32)
